# revision 1
# baseline (speedup 1.0000x reference)
"""Trainium2 Bass kernel for CliffordAdaptiveGraphAttention.

Math restructuring (validated numerically against the reference):
  * The "clifford enhancement" term `enh` is added uniformly to all 8 head
    scores, so it cancels in softmax -> the geometric/wedge product tables are
    only needed through the *symmetrized* geometric product that produces the
    inner product `ip = 0.5*(gp(sa,da) + gp(da,sa)) = sa^T Gsym da`.
  * Work in a permuted Clifford basis (grade-ordered: scalar, 4 vectors, ...)
    so the vector components sit in contiguous slots 1..4. Weights/tables are
    permuted host-side.
  * mp2 columns are permuted into an L^T layout (col j*4+i = L[i,j], upper
    triangle dropped/zeroed) so the Cholesky factor comes out of the matmul
    directly; softplus is applied to the 4 diagonal slots (stride-5 AP).
  * metric @ v is evaluated as L (L^T v) per edge with free-dim broadcast ops.

Layout strategy per 512-edge chunk (per core):
  * Inputs DMA'd in natural [128b, 4, 128f] tiles, PE-transposed to
    feature-major [128f, 512b]; the whole linear chain runs feature-major so
    every matmul has batch(=512) as the moving dimension.
  * Small per-edge 4x4 ops run batch-major on tiny [128, 4, 16] tiles
    (PE-transposed over, cheap: 16-col transposes).
  * P = sa (x) da outer products computed batch-major, transposed back, and
    contracted with the packed Gsym table on the PE.
  * Softmax runs feature-major using ones/block matmuls for the partition
    reductions; normalized weights are broadcast across head dims with an
    expansion matmul.
"""

import numpy as np

import bass_rust as _bass_rust
import concourse.bass as bass
import concourse.mybir as mybir
from concourse.alu_op_type import AluOpType
from concourse.bass import ts
from concourse.tile import TileContext
from concourse.bass_utils import run_bass_kernel_spmd

F32 = mybir.dt.float32
FR = mybir.dt.float32r
AF = mybir.ActivationFunctionType

N_CORES = 8
B_FULL = 262144
D = 128
CH = 512          # edges per chunk
SUB = CH // 128   # 128-row subtiles per chunk

# grade-ordered Clifford basis permutation for Cl(4,0)
PERM = np.array([0, 1, 2, 4, 8, 3, 5, 6, 9, 10, 12, 7, 11, 13, 14, 15])

# ---- consts layout (columns of the packed [128, NCOLS] constants array) ----
OFF_ID = 0
OFF_WNP = 128
OFF_WEP = 256
OFF_WI1A = 384
OFF_WI1B = 512
OFF_WI2 = 640
OFF_WMP1 = 768
OFF_WQ = 896
OFF_WK = 1024
OFF_WV = 1152
OFF_WOUT = 1280
OFF_WMP2 = 1408
OFF_WNTC = 1440
OFF_GA = 1472
OFF_GB = 1488
OFF_WCTO = 1504
OFF_EXP8 = 1632
OFF_BLK = 1760
OFF_ONES = 1768
OFF_BIAS = 1776
# bias cols: 0 np, 1 ep, 2 im1, 3 im2, 4 mp1, 5 q, 6 k, 7 v, 8 outc,
#            9 mp2pp(16), 10 ntc2(32)
OFF_SPC = OFF_BIAS + 11   # 7 softplus polynomial coeff columns (c0..c6)
NCOLS = OFF_SPC + 7


def _geo_table():
    N = 4
    BASIS = 16
    geo = np.zeros((BASIS, BASIS, BASIS), np.float64)
    for i in range(BASIS):
        bi = [(i >> (N - 1 - k)) & 1 for k in range(N)]
        for j in range(BASIS):
            bj = [(j >> (N - 1 - k)) & 1 for k in range(N)]
            sign = 1.0
            for k in range(N):
                if bj[k] and sum(bi[:k]) % 2 == 1:
                    sign = -sign
            geo[i, j, i ^ j] = sign
    return geo


def _pack_consts(w):
    """Build the [128, NCOLS] fp32 constants array from the weight dict."""
    C = np.zeros((128, NCOLS), np.float32)
    C[:, OFF_ID:OFF_ID + 128] = np.eye(128, dtype=np.float32)
    # first two linear layers fused: relu(src@A + edge@B + b') with
    # A = np_W @ im1_W[:128], B = ep_W @ im1_W[128:]
    im1a = w["im1_W"][:128].astype(np.float64)
    im1b = w["im1_W"][128:].astype(np.float64)
    C[:, OFF_WNP:OFF_WNP + 128] = (w["np_W"].astype(np.float64) @ im1a)
    C[:, OFF_WEP:OFF_WEP + 128] = (w["ep_W"].astype(np.float64) @ im1b)
    C[:, OFF_WI2:OFF_WI2 + 128] = w["im2_W"]
    C[:, OFF_WMP1:OFF_WMP1 + 128] = w["mp1_W"]
    C[:, OFF_WQ:OFF_WQ + 128] = w["q_W"]
    C[:, OFF_WK:OFF_WK + 128] = w["k_W"]
    C[:, OFF_WV:OFF_WV + 128] = w["v_W"]
    C[:, OFF_WOUT:OFF_WOUT + 128] = w["out_W"]

    # mp2 in L^T ("Lt") layout: col j*4+i <- m[i,j] for i>=j, else zero
    W16 = np.zeros((128, 16), np.float32)
    b16 = np.zeros(16, np.float32)
    for j in range(4):
        for i in range(4):
            if i >= j:
                W16[:, j * 4 + i] = w["mp2_W"][:, i * 4 + j]
                b16[j * 4 + i] = w["mp2_b"][i * 4 + j]
    C[:, OFF_WMP2:OFF_WMP2 + 16] = W16   # cols 16:32 stay zero (padding)

    ntc_Wp = w["ntc_W"][:, PERM]
    ntc_bp = w["ntc_b"][PERM]
    C[:, OFF_WNTC:OFF_WNTC + 16] = ntc_Wp  # cols 16:32 stay zero (padding)

    geo = _geo_table()
    gsym = 0.5 * (geo + np.swapaxes(geo, 0, 1))
    gsym_p = gsym[np.ix_(PERM, PERM, PERM)].reshape(256, 16).astype(np.float32)
    C[:, OFF_GA:OFF_GA + 16] = gsym_p[:128]
    C[:, OFF_GB:OFF_GB + 16] = gsym_p[128:]

    C[:16, OFF_WCTO:OFF_WCTO + 128] = w["cto_W"][PERM, :]

    # expand8[h, hd] = 1 if hd//16 == h  (lhsT for head-broadcast matmul)
    exp8 = np.zeros((8, 128), np.float32)
    for h in range(8):
        exp8[h, h * 16:(h + 1) * 16] = 1.0
    C[:8, OFF_EXP8:OFF_EXP8 + 128] = exp8
    # blockones16[hd, h] = 1 if hd//16 == h  (lhsT for per-head reduction)
    C[:, OFF_BLK:OFF_BLK + 8] = exp8.T
    C[:, OFF_ONES:OFF_ONES + 8] = 1.0

    C[:, OFF_BIAS + 0] = w["np_b"]
    C[:, OFF_BIAS + 1] = w["ep_b"]
    C[:, OFF_BIAS + 2] = (w["np_b"].astype(np.float64) @ im1a
                          + w["ep_b"].astype(np.float64) @ im1b
                          + w["im1_b"])
    C[:, OFF_BIAS + 3] = w["im2_b"]
    C[:, OFF_BIAS + 4] = w["mp1_b"]
    C[:, OFF_BIAS + 5] = w["q_b"]
    C[:, OFF_BIAS + 6] = w["k_b"]
    C[:, OFF_BIAS + 7] = w["v_b"]
    C[:, OFF_BIAS + 8] = w["out_b"] + w["cto_b"]
    C[:16, OFF_BIAS + 9] = b16
    C[:16, OFF_BIAS + 10] = ntc_bp

    # softplus-on-[-1,1] polynomial coefficients, replicated per partition
    xs = np.linspace(-1.0, 1.0, 20001)
    cheb = np.polynomial.chebyshev.Chebyshev.fit(xs, np.log1p(np.exp(xs)), 6)
    spc = cheb.convert(kind=np.polynomial.Polynomial).coef
    for k in range(7):
        C[:, OFF_SPC + k] = spc[k]
    return C


def build_kernel(n_edges, mm_dtype=mybir.dt.float32r, stage=6, reps=1):
    """Build the per-core Bass program: n_edges rows -> n_edges rows.

    Software-pipelined: chunk c's serial tail (metric/P/output) is emitted
    around chunk c+1's head (loads/attention/MLP) so the long dependency
    chain overlaps the wide matmul work of the next chunk.
    """
    assert n_edges % CH == 0
    n_chunks = n_edges // CH

    nc = bass.Bass()
    src_d = nc.dram_tensor("src", [n_edges, D], F32, kind="ExternalInput")
    dst_d = nc.dram_tensor("dst", [n_edges, D], F32, kind="ExternalInput")
    edg_d = nc.dram_tensor("edg", [n_edges, D], F32, kind="ExternalInput")
    cst_d = nc.dram_tensor("consts", [128, NCOLS], F32, kind="ExternalInput")
    out_d = nc.dram_tensor("out", [n_edges, D], F32, kind="ExternalOutput")

    MMD = mm_dtype   # dtype for matmul operand tiles

    with TileContext(nc) as tc, nc.allow_low_precision(
            reason="fp32r matmul operand rounding"):
        with (
            tc.tile_pool(name="cst", bufs=1) as cstp,
            tc.tile_pool(name="inn", bufs=3) as innp,
            tc.tile_pool(name="fmt", bufs=3) as fmtp,
            tc.tile_pool(name="act", bufs=3) as actp,
            tc.tile_pool(name="big", bufs=3) as bigp,
            tc.tile_pool(name="sml", bufs=3) as smlp,
            tc.tile_pool(name="ptin", bufs=2, space="PSUM") as ptin,
            tc.tile_pool(name="psm", bufs=2, space="PSUM") as psm,
            tc.tile_pool(name="pss", bufs=2, space="PSUM") as pss,
            tc.tile_pool(name="psb", bufs=1, space="PSUM") as psb,
            tc.tile_pool(name="ptp", bufs=1, space="PSUM") as ptp,
        ):
            cst = cstp.tile([128, NCOLS], F32)
            nc.sync.dma_start(out=cst[:], in_=cst_d[:])

            ident = cst[:, OFF_ID:OFF_ID + 128]

            # one-time rounding copy of all matmul weights/tables to fp32r
            cstr = cstp.tile([128, OFF_BIAS - 128], MMD, tag="cstr")
            nc.vector.tensor_copy(cstr[:], cst[:, 128:OFF_BIAS])

            def W(off, n=128, p=128):
                return cstr[:p, off - 128:off - 128 + n]

            def bias(j, p=128):
                return cst[:p, OFF_BIAS + j:OFF_BIAS + j + 1]

            def emit_head(c):
                row0 = c * CH
                st = {}
                fm = {}
                for name, dram in (("src", src_d), ("dst", dst_d),
                                   ("edg", edg_d)):
                    nat = innp.tile([128, SUB, 128], F32, tag=f"in_{name}")
                    nc.sync.dma_start(
                        out=nat[:],
                        in_=dram[row0:row0 + CH, :].rearrange(
                            "(n p) f -> p n f", p=128))
                    pt = ptin.tile([128, CH], F32, tag="tin")
                    for n in range(SUB):
                        nc.tensor.transpose(pt[:, ts(n, 128)], nat[:, n, :],
                                            ident)
                    t = fmtp.tile([128, CH], MMD, tag=f"fm_{name}")
                    if name == "dst":
                        nc.scalar.copy(t[:], pt[:])
                    else:
                        nc.vector.tensor_copy(t[:], pt[:])
                    fm[name] = t

                # attention matmuls (independent of the MLP chain)
                pq = psm.tile([128, CH], F32, tag="mm")
                nc.tensor.matmul(pq[:], W(OFF_WQ), fm["src"][:], start=True,
                                 stop=True)
                pk = psm.tile([128, CH], F32, tag="mm")
                nc.tensor.matmul(pk[:], W(OFF_WK), fm["dst"][:], start=True,
                                 stop=True)
                ksb = bigp.tile([128, CH], F32, tag="ksb")
                nc.scalar.activation(ksb[:], pk[:], AF.Identity, bias=bias(6))

                # metric MLP layer 1 (np/ep fused into im1)
                ps1 = psm.tile([128, CH], F32, tag="mm")
                nc.tensor.matmul(ps1[:], W(OFF_WNP), fm["src"][:], start=True,
                                 stop=False)
                nc.tensor.matmul(ps1[:], W(OFF_WEP), fm["edg"][:], start=False,
                                 stop=True)
                i1 = actp.tile([128, CH], MMD, tag="i1")
                nc.scalar.activation(i1[:], ps1[:], AF.Relu, bias=bias(2))

                pv = psm.tile([128, CH], F32, tag="mm")
                nc.tensor.matmul(pv[:], W(OFF_WV), fm["dst"][:], start=True,
                                 stop=True)
                vsb = bigp.tile([128, CH], F32, tag="vsb")
                nc.scalar.activation(vsb[:], pv[:], AF.Identity, bias=bias(7))
                prod = bigp.tile([128, CH], MMD, tag="prod")
                nc.vector.scalar_tensor_tensor(
                    prod[:], pq[:], bias(5), ksb[:],
                    op0=AluOpType.add, op1=AluOpType.mult)
                psc = pss.tile([8, CH], F32, tag="sm")
                nc.tensor.matmul(psc[:], W(OFF_BLK, n=8), prod[:],
                                 start=True, stop=True)
                e = smlp.tile([8, CH], MMD, tag="e")
                nc.scalar.activation(e[:], psc[:], AF.Exp, scale=0.25)

                ps2 = psm.tile([128, CH], F32, tag="mm")
                nc.tensor.matmul(ps2[:], W(OFF_WI2), i1[:], start=True,
                                 stop=True)
                i2 = actp.tile([128, CH], MMD, tag="i2")
                nc.scalar.activation(i2[:], ps2[:], AF.Relu, bias=bias(3))

                pden = pss.tile([1, CH], F32, tag="sm")
                nc.tensor.matmul(pden[:], W(OFF_ONES, n=1, p=8), e[:],
                                 start=True, stop=True)
                rden = smlp.tile([1, CH], MMD, tag="rden")
                nc.vector.reciprocal(rden[:], pden[:])
                prd8 = pss.tile([8, CH], F32, tag="sm")
                nc.tensor.matmul(prd8[:], W(OFF_ONES, n=8, p=1), rden[:],
                                 start=True, stop=True)
                wsb = smlp.tile([8, CH], MMD, tag="w")
                nc.vector.tensor_mul(wsb[:], e[:], prd8[:])

                ps3 = psm.tile([128, CH], F32, tag="mm")
                nc.tensor.matmul(ps3[:], W(OFF_WMP1), i2[:], start=True,
                                 stop=True)
                m1 = actp.tile([128, CH], MMD, tag="m1")
                nc.scalar.activation(m1[:], ps3[:], AF.Relu, bias=bias(4))

                pwr = psm.tile([128, CH], F32, tag="mm")
                nc.tensor.matmul(pwr[:], W(OFF_EXP8, p=8), wsb[:],
                                 start=True, stop=True)
                att = bigp.tile([128, CH], MMD, tag="att")
                nc.vector.tensor_mul(att[:], vsb[:], pwr[:])
                st["att"] = att
                st["fm"] = fm

                # mp2 + ntc heads
                pmt = pss.tile([16, CH], F32, tag="sm")
                nc.tensor.matmul(pmt[:], W(OFF_WMP2, n=16), m1[:],
                                 start=True, stop=True)
                mt = smlp.tile([16, CH], F32, tag="mt")
                nc.scalar.activation(mt[:], pmt[:], AF.Tanh, bias=bias(9, p=16))
                pnt1 = pss.tile([16, CH], F32, tag="sm")
                nc.tensor.matmul(pnt1[:], W(OFF_WNTC, n=16),
                                 fm["src"][:], start=True, stop=True)
                sct = smlp.tile([16, CH], F32, tag="sct")
                nc.scalar.activation(sct[:], pnt1[:], AF.Identity,
                                     bias=bias(10, p=16))
                pnt2 = pss.tile([16, CH], F32, tag="sm")
                nc.tensor.matmul(pnt2[:], W(OFF_WNTC, n=16),
                                 fm["dst"][:], start=True, stop=True)
                dct = smlp.tile([16, CH], F32, tag="dct")
                nc.vector.tensor_scalar(dct[:], pnt2[:], bias(10, p=16), 0.0,
                                        op0=AluOpType.add, op1=AluOpType.bypass)
                st["mt"], st["sct"], st["dct"] = mt, sct, dct
                return st

            def emit_tail1(c, st):
                mt, sct, dct = st["mt"], st["sct"], st["dct"]
                bm = bigp.tile([128, SUB, 48], F32, tag="bm")
                for n in range(SUB):
                    pbm = psb.tile([128, 3, 128], F32, tag="pbm")
                    nc.tensor.transpose(pbm[:, 0, 0:16], mt[:, ts(n, 128)],
                                        ident[:16, :16])
                    nc.tensor.transpose(pbm[:, 1, 0:16], sct[:, ts(n, 128)],
                                        ident[:16, :16])
                    nc.tensor.transpose(pbm[:, 2, 0:16], dct[:, ts(n, 128)],
                                        ident[:16, :16])
                    nc.vector.tensor_copy(
                        bm[:, n, :].rearrange("p (t c) -> p t c", t=3),
                        pbm[:, :, 0:16])

                # softplus on the tanh-bounded diagonal via a degree-6
                # polynomial (Horner, gpsimd tensor_tensor only) so the ACT
                # engine never needs the Ln/Softplus tables.
                dg = bm[:, :, 0:16:5]

                def spcb(k):
                    return cst[:, OFF_SPC + k:OFF_SPC + k + 1] \
                        .unsqueeze(1).broadcast_to((128, SUB, 4))

                u = bigp.tile([128, SUB, 4], F32, tag="spt")
                nc.gpsimd.tensor_tensor(u[:], dg, spcb(6), op=AluOpType.mult)
                for k in (5, 4, 3, 2, 1):
                    nc.gpsimd.tensor_tensor(u[:], u[:], spcb(k),
                                            op=AluOpType.add)
                    nc.gpsimd.tensor_tensor(u[:], u[:], dg,
                                            op=AluOpType.mult)
                nc.gpsimd.tensor_tensor(dg, u[:], spcb(0), op=AluOpType.add)

                # adaptive metric: v <- L (L^T v), in place
                lt = bm[:, :, 0:16].rearrange("p n (j i) -> p n j i", j=4)
                for off in (16, 32):
                    v = bm[:, :, off + 1:off + 5]
                    tmp1 = bigp.tile([128, SUB, 4, 4], F32, tag="tmp1")
                    nc.gpsimd.tensor_tensor(
                        tmp1[:], lt,
                        v.unsqueeze(2).broadcast_to((128, SUB, 4, 4)),
                        op=AluOpType.mult)
                    t4 = bigp.tile([128, SUB, 4], F32, tag="t4")
                    nc.vector.reduce_sum(t4[:], tmp1[:],
                                         axis=mybir.AxisListType.X)
                    tmp2 = bigp.tile([128, SUB, 4, 4], F32, tag="tmp1")
                    nc.gpsimd.tensor_tensor(
                        tmp2[:], lt,
                        t4[:].unsqueeze(3).broadcast_to((128, SUB, 4, 4)),
                        op=AluOpType.mult)
                    u4 = bigp.tile([128, SUB, 4], F32, tag="t4")
                    nc.vector.reduce_sum(u4[:], tmp2[:].transpose([0, 1, 3, 2]),
                                         axis=mybir.AxisListType.X)
                    nc.gpsimd.tensor_copy(v, u4[:])

                # P = sa (x) da (batch-major)
                pP = bigp.tile([128, SUB, 256], F32, tag="pP")
                sa = bm[:, :, 16:32]
                da = bm[:, :, 32:48]
                nc.gpsimd.tensor_tensor(
                    pP[:],
                    sa.unsqueeze(3).broadcast_to((128, SUB, 16, 16)),
                    da.unsqueeze(2).broadcast_to((128, SUB, 16, 16)),
                    op=AluOpType.mult)
                st["pP"] = pP

            def emit_tail2(c, st):
                row0 = c * CH
                pP, att = st["pP"], st["att"]
                pip = pss.tile([16, CH], F32, tag="sm")

                pfa = ptp.tile([128, CH], F32, tag="tp")
                for n in range(SUB):
                    nc.tensor.transpose(pfa[:, ts(n, 128)], pP[:, n, 0:128],
                                        ident)
                Pa = fmtp.tile([128, CH], MMD, tag="Pa")
                nc.scalar.copy(Pa[:], pfa[:])
                nc.tensor.matmul(pip[:], W(OFF_GA, n=16), Pa[:],
                                 start=True, stop=False)

                pfb = ptp.tile([128, CH], F32, tag="tp")
                for n in range(SUB):
                    nc.tensor.transpose(pfb[:, ts(n, 128)], pP[:, n, 128:256],
                                        ident)
                Pb = fmtp.tile([128, CH], MMD, tag="Pb")
                nc.vector.tensor_copy(Pb[:], pfb[:])
                nc.tensor.matmul(pip[:], W(OFF_GB, n=16), Pb[:],
                                 start=False, stop=True)
                ip = smlp.tile([16, CH], MMD, tag="ip")
                nc.vector.tensor_copy(ip[:], pip[:])

                po = psm.tile([128, CH], F32, tag="mm")
                nc.tensor.matmul(po[:], W(OFF_WOUT), att[:], start=True,
                                 stop=False)
                nc.tensor.matmul(po[:], W(OFF_WCTO, p=16), ip[:],
                                 start=False, stop=True)
                of = bigp.tile([128, CH], F32, tag="of")
                nc.scalar.activation(of[:], po[:], AF.Identity, bias=bias(8))

                pot = ptp.tile([128, SUB, 128], F32, tag="tp")
                for n in range(SUB):
                    nc.tensor.transpose(pot[:, n, :], of[:, ts(n, 128)], ident)
                ot = bigp.tile([128, SUB, 128], F32, tag="ot")
                nc.vector.tensor_copy(ot[:], pot[:])
                nc.sync.dma_start(
                    out=out_d[row0:row0 + CH, :].rearrange(
                        "(n p) f -> p n f", p=128),
                    in_=ot[:])

            prev = None
            prev_c = None
            for i in range(n_chunks * reps):
                c = i % n_chunks
                st = emit_head(c)
                emit_tail1(c, st)
                if prev is not None:
                    emit_tail2(prev_c, prev)
                prev, prev_c = st, c
            emit_tail2(prev_c, prev)

    # TRN2 allows at most one semaphore wait per engine instruction; split
    # multi-wait instructions through event semaphores (same pass bacc runs).
    _bass_rust.generate_event_semaphores(nc)
    return nc


_CACHE = {}


def _get_kernel(n_edges, mm_dtype, stage=6):
    key = (n_edges, mm_dtype, stage)
    if key not in _CACHE:
        _CACHE[key] = build_kernel(n_edges, mm_dtype, stage)
    return _CACHE[key]


def kernel(**inputs):
    w = {k: np.asarray(v, np.float32) for k, v in inputs.items()
         if k not in ("src_nodes", "dst_nodes", "edge_features", "edge_indices")}
    src = np.ascontiguousarray(np.asarray(inputs["src_nodes"], np.float32))
    dst = np.ascontiguousarray(np.asarray(inputs["dst_nodes"], np.float32))
    edg = np.ascontiguousarray(np.asarray(inputs["edge_features"], np.float32))

    B = src.shape[0]
    assert B % N_CORES == 0
    bc = B // N_CORES

    consts = _pack_consts(w)
    nc = _get_kernel(bc, mybir.dt.float32r)

    in_maps = []
    for i in range(N_CORES):
        sl = slice(i * bc, (i + 1) * bc)
        in_maps.append({
            "src": src[sl], "dst": dst[sl], "edg": edg[sl], "consts": consts,
        })
    res = run_bass_kernel_spmd(nc, in_maps, list(range(N_CORES)))
    return np.concatenate([res.results[i]["out"] for i in range(N_CORES)], axis=0)

